# revision 2
# baseline (speedup 1.0000x reference)
"""Trainium2 Bass kernel for nn_Minerva_37211596652565 (retrieval_knn).

Computes, for X[8192,128], D[16384,128], r[16384,128] (all f32):
    Xn = l2norm_rows(X); Dn = l2norm_rows(D)
    a  = Xn @ Dn.T                     # [N, M] cosine sims
    a  = sign(a)*|a|^3  ==  a**3       # odd power => plain cube
    echo = a @ r                       # [N, 128]

Sharding: data-parallel over X rows across 8 NeuronCores (N_loc=1024 each);
D and r replicated per core. No collectives.

Per-core dataflow (everything f32 in, bf16 matmul operands):
  - load all of D and r into SBUF ([128, 128, 128] each, 8 MB)
  - ss_m = sum_d D^2 via ACT Square+accum;  s = 1/sqrt(ss)  (Sqrt + DVE recip)
  - X normalized on-chip, transposed via PE into XT_bf16 [d=128, n=1024]
  - per m-tile t (128 of them):
      PE  transpose D-tile -> psum; ACT copy-cast -> DT_bf16 [d,m]
      PE  mm1: aT[m,n] += DT_bf16.T @ XT_bf16   (raw-D cosine numerators)
      DVE fused custom op: a3 = (s_m * aT)^3 -> bf16   (single pass)
      ACT copy-cast r-tile -> bf16
      PE  mm2: echoT[k,n] += r_bf16.T @ a3      (psum accumulation over t)
  - epilogue: echoT -> transpose via PE -> OUT [n,k]
"""

import sys

sys.path.insert(0, "/opt/trn_rl_repo")

import numpy as np

import concourse.bacc as bacc
import concourse.bass as bass
import concourse.tile as tile
from concourse import mybir
from concourse.bass_utils import run_bass_kernel_spmd
from concourse.masks import make_identity
from concourse.bass import ts

# ----------------------------------------------------------------------------
# Custom DVE op: out = (in0 * s0)^3, s0 a per-partition [P,1] scalar.
# One streaming DVE pass (3 ALU stages), replaces ACT-square + DVE-mult.
# ----------------------------------------------------------------------------
from concourse import dve_ops as dvo
from concourse.dve_spec import Spec, Src0, C0, sq, lower, _has_src1
from concourse.dve_uop import DveOpSpec


def _register_cube_op():
    name = "CUBE_SCALED_ANT"
    for op in dvo.OPS:
        if op.name == name:
            return op
    t = Src0 * C0
    spec = Spec(
        body=t * sq(t),
        reference=lambda in0, in1, s0, s1, imm2: (in0.astype(np.float32) * s0) ** 3,
    )
    row = max(dvo._SUB_OPCODE_FOR_NAME.values()) + 1
    assert row < 0x20
    dvo._SUB_OPCODE_FOR_NAME[name] = row
    shas = {}
    for ver in ("v3", "v4"):
        uops = lower(spec, ver=ver)
        shas[ver] = DveOpSpec(
            name=name, opcode=row, uops=uops, rd1_en=_has_src1(spec)
        ).sha(ver)
    op = dvo.DveOp(name, spec, subdim=False, uops_sha=shas)
    dvo.OPS.append(op)
    dvo.CUSTOM_DVE_SPECS[name] = spec
    return op


CUBE_OP = _register_cube_op()

# Problem shapes (hardcoded per contract).
N, M, d = 8192, 16384, 128
NCORES = 8
N_LOC = N // NCORES  # 1024
P = 128
NT = N_LOC // P  # 8 n-tiles per core
MT = M // P  # 128 m-tiles

F32 = mybir.dt.float32
BF16 = mybir.dt.bfloat16


def build_kernel(nc: bass.Bass, Xap, Dap, Rap, OUTap, tc: tile.TileContext):
    from contextlib import ExitStack

    with ExitStack() as ctx:
        consts = ctx.enter_context(tc.tile_pool(name="consts", bufs=1))
        big = ctx.enter_context(tc.tile_pool(name="big", bufs=1))
        dtp = ctx.enter_context(tc.tile_pool(name="dtp", bufs=3))
        rbfp = ctx.enter_context(tc.tile_pool(name="rbfp", bufs=3))
        a3p = ctx.enter_context(tc.tile_pool(name="a3p", bufs=3))
        scrapp = ctx.enter_context(tc.tile_pool(name="scrapp", bufs=2))
        xnp = ctx.enter_context(tc.tile_pool(name="xnp", bufs=2))
        outp = ctx.enter_context(tc.tile_pool(name="outp", bufs=2))

        pa = ctx.enter_context(tc.tile_pool(name="pa", bufs=2, space="PSUM"))
        pecho = ctx.enter_context(tc.tile_pool(name="pecho", bufs=1, space="PSUM"))
        pt = ctx.enter_context(tc.tile_pool(name="pt", bufs=2, space="PSUM"))
        ident = consts.tile([P, P], F32)
        make_identity(nc, ident)

        # ---- bulk loads: D, r fully resident in SBUF --------------------
        Dbuf = big.tile([P, MT, d], F32)  # Dbuf[p, t, :] = D[t*128+p, :]
        Rbuf = big.tile([P, MT, d], F32)
        Dr = Dap.rearrange("(t p) d -> p t d", p=P)
        Rr = Rap.rearrange("(t p) d -> p t d", p=P)
        CH = 16  # m-tiles per DMA (1 MB chunks)
        for c in range(0, MT, CH):
            nc.sync.dma_start(out=Dbuf[:, c : c + CH, :], in_=Dr[:, c : c + CH, :])
            nc.sync.dma_start(out=Rbuf[:, c : c + CH, :], in_=Rr[:, c : c + CH, :])

        Xbuf = big.tile([P, NT, d], F32)
        Xr = Xap.rearrange("(i p) d -> p i d", p=P)
        nc.sync.dma_start(out=Xbuf[:, :, :], in_=Xr[:, :, :])

        # ---- X prep: normalize rows, transpose into XT_bf16 [d, n_loc] --
        ssx = consts.tile([P, NT], F32)
        sx = consts.tile([P, NT], F32)
        xscrap = scrapp.tile([P, d], BF16, tag="xscrap")
        for i in range(NT):
            nc.scalar.activation(
                out=xscrap,
                in_=Xbuf[:, i, :],
                func=mybir.ActivationFunctionType.Square,
                accum_out=ssx[:, i : i + 1],
            )
        nc.scalar.activation(
            out=sx, in_=ssx, func=mybir.ActivationFunctionType.Sqrt
        )
        nc.vector.reciprocal(out=sx, in_=sx)

        XT = consts.tile([P, N_LOC], BF16)  # [d, n]
        for i in range(NT):
            xn = xnp.tile([P, d], F32, tag="xn")
            nc.vector.tensor_scalar_mul(xn, Xbuf[:, i, :], sx[:, i : i + 1])
            ptile = pt.tile([P, P], F32, tag="pt")
            nc.tensor.transpose(ptile, xn, ident)
            nc.scalar.activation(
                out=XT[:, ts(i, P)],
                in_=ptile,
                func=mybir.ActivationFunctionType.Copy,
            )

        # ---- D row norms: ss = sum_d D^2 ; s = 1/sqrt(ss) ---------------
        ssd = consts.tile([P, MT], F32)
        sd = consts.tile([P, MT], F32)
        for t in range(MT):
            dscrap = scrapp.tile([P, d], BF16, tag="dscrap")
            nc.scalar.activation(
                out=dscrap,
                in_=Dbuf[:, t, :],
                func=mybir.ActivationFunctionType.Square,
                accum_out=ssd[:, t : t + 1],
            )
        nc.scalar.activation(
            out=sd, in_=ssd, func=mybir.ActivationFunctionType.Sqrt
        )
        nc.vector.reciprocal(out=sd, in_=sd)

        # ---- main loop over m-tiles -------------------------------------
        echoT = pecho.tile([P, N_LOC], F32)  # [k, n] psum accumulator
        for t in range(MT):
            # transpose D tile -> [d, m] bf16
            ptile = pt.tile([P, P], F32, tag="pt")
            nc.tensor.transpose(ptile, Dbuf[:, t, :], ident)
            dtb = dtp.tile([P, P], BF16, tag="dtb")
            nc.scalar.activation(
                out=dtb, in_=ptile, func=mybir.ActivationFunctionType.Copy
            )

            # mm1: aT[m, n] = D_tile @ Xn^T   (raw D)
            aT = pa.tile([P, N_LOC], F32, tag="aT")
            for c in range(N_LOC // 512):
                nc.tensor.matmul(
                    aT[:, ts(c, 512)],
                    lhsT=dtb,
                    rhs=XT[:, ts(c, 512)],
                    start=True,
                    stop=True,
                )

            # fused cube with per-partition norm scale: a3 = (s_m * aT)^3
            a3 = a3p.tile([P, N_LOC], BF16, tag="a3")
            nc.vector._custom_dve(
                CUBE_OP, out=a3, in0=aT, s0=sd[:, t : t + 1]
            )

            # r tile -> bf16
            rbf = rbfp.tile([P, P], BF16, tag="rbf")
            nc.scalar.activation(
                out=rbf, in_=Rbuf[:, t, :], func=mybir.ActivationFunctionType.Copy
            )

            # mm2: echoT[k, n] += r_tile^T @ a3
            for c in range(N_LOC // 512):
                nc.tensor.matmul(
                    echoT[:, ts(c, 512)],
                    lhsT=rbf,
                    rhs=a3[:, ts(c, 512)],
                    start=(t == 0),
                    stop=(t == MT - 1),
                )

        # ---- epilogue: transpose echoT -> OUT [n, k] --------------------
        echoS = consts.tile([P, N_LOC], F32)
        nc.scalar.activation(
            out=echoS, in_=echoT, func=mybir.ActivationFunctionType.Copy
        )
        for i in range(NT):
            ptile = pt.tile([P, P], F32, tag="pt")
            nc.tensor.transpose(ptile, echoS[:, ts(i, P)], ident)
            otile = outp.tile([P, P], F32, tag="otile")
            nc.vector.tensor_copy(otile, ptile)
            nc.sync.dma_start(out=OUTap[ts(i, P), :], in_=otile)


_COMPILED = None


def _get_compiled():
    global _COMPILED
    if _COMPILED is None:
        nc = bacc.Bacc(
            "TRN2",
            target_bir_lowering=False,
            debug=False,
            num_devices=1,
        )
        Xap = nc.dram_tensor("X", [N_LOC, d], F32, kind="ExternalInput").ap()
        Dap = nc.dram_tensor("D", [M, d], F32, kind="ExternalInput").ap()
        Rap = nc.dram_tensor("R", [M, d], F32, kind="ExternalInput").ap()
        OUTap = nc.dram_tensor("OUT", [N_LOC, d], F32, kind="ExternalOutput").ap()
        with tile.TileContext(nc) as tc:
            build_kernel(nc, Xap, Dap, Rap, OUTap, tc)
        nc.compile()
        _COMPILED = nc
    return _COMPILED


def kernel(X, D, r, _trace=False, _trace_kwargs=None):
    X = np.ascontiguousarray(np.asarray(X), dtype=np.float32)
    D = np.ascontiguousarray(np.asarray(D), dtype=np.float32)
    r = np.ascontiguousarray(np.asarray(r), dtype=np.float32)
    assert X.shape == (N, d) and D.shape == (M, d) and r.shape == (M, d)

    nc = _get_compiled()
    in_maps = [
        {
            "X": np.ascontiguousarray(X[c * N_LOC : (c + 1) * N_LOC]),
            "D": D,
            "R": r,
        }
        for c in range(NCORES)
    ]
    res = run_bass_kernel_spmd(
        nc,
        in_maps,
        core_ids=list(range(NCORES)),
        trace=_trace,
        **(_trace_kwargs or {}),
    )
    out = np.concatenate([res.results[c]["OUT"] for c in range(NCORES)], axis=0)
    if _trace:
        kernel._last_results = res
    return out


# revision 7
# speedup vs baseline: 1.0333x; 1.0333x over previous
"""Trainium2 Bass kernel for nn_Minerva_37211596652565 (retrieval_knn).

reference:
    Xn = l2norm_rows(X); Dn = l2norm_rows(D)
    a  = Xn @ Dn.T            # [N, M] cosine sims
    a  = sign(a)*|a|^3 == a^3 # odd power => plain cube
    echo = a @ r              # [N, 128]

Sharding: data-parallel over X rows across 8 NeuronCores (N_loc=1024/core),
D and r replicated. No collectives.

Host-side layout prep (pure data movement, no math):
    DT     = D.T  (contiguous [128, M])     -> mm1 stationary needs d-major
    r_perm = tile-permuted r so each SBUF partition gets a contiguous 64KB run

Per-core dataflow:
    - DT, r fully resident in SBUF; X normalized+transposed on chip (f32)
    - ss_m = sum_d D^2:  ACT Square(DT chunk)->DTsq,  PE ones-matmul -> psum col
      s = 1/sqrt(ss) per 16-tile group (ACT Sqrt + DVE reciprocal)
    - per m-tile t (128):
        PE  mm1 (f32r): aT[m,n] = DT_tile.T @ XT       (raw-D numerators)
        DVE fused custom op: a3 = (s_m * aT)^3 -> bf16 (single pass from PSUM)
        ACT copy-cast r-tile -> bf16
        PE  mm2 (bf16): echoT[k,n] += r_tile.T @ a3    (PSUM accum over t)
    - epilogue: echoT --PE transpose--> OUT[n,k]
"""

import sys

sys.path.insert(0, "/opt/trn_rl_repo")

import numpy as np

import concourse.bacc as bacc
import concourse.bass as bass
import concourse.tile as tile
from concourse import mybir
from concourse.bass_utils import run_bass_kernel_spmd
from concourse.masks import make_identity
from concourse.bass import ts

# ----------------------------------------------------------------------------
# Custom DVE op: out = (in0 * s0)^3, s0 a per-partition [P,1] scalar.
# One streaming DVE pass (3 ALU stages) replaces ACT-square + DVE-mult.
# ----------------------------------------------------------------------------
from concourse import dve_ops as dvo
from concourse.dve_spec import Spec, Src0, C0, sq, lower, _has_src1
from concourse.dve_uop import DveOpSpec


def _register_cube_op():
    name = "CUBE_SCALED_ANT"
    for op in dvo.OPS:
        if op.name == name:
            return op
    t = Src0 * C0
    spec = Spec(
        body=t * sq(t),
        reference=lambda in0, in1, s0, s1, imm2: (in0.astype(np.float32) * s0) ** 3,
    )
    row = max(dvo._SUB_OPCODE_FOR_NAME.values()) + 1
    assert row < 0x20
    dvo._SUB_OPCODE_FOR_NAME[name] = row
    shas = {}
    for ver in ("v3", "v4"):
        uops = lower(spec, ver=ver)
        shas[ver] = DveOpSpec(
            name=name, opcode=row, uops=uops, rd1_en=_has_src1(spec)
        ).sha(ver)
    op = dvo.DveOp(name, spec, subdim=False, uops_sha=shas)
    dvo.OPS.append(op)
    dvo.CUSTOM_DVE_SPECS[name] = spec
    return op


CUBE_OP = _register_cube_op()

# Problem shapes (hardcoded per contract).
N, M, d = 8192, 16384, 128
NCORES = 8
N_LOC = N // NCORES  # 1024
P = 128
NT = N_LOC // P  # 8 n-tiles per core
MT = M // P  # 128 m-tiles
GRP = 16  # m-tiles per sqrt/recip group
F32 = mybir.dt.float32
F32R = mybir.dt.float32r
BF16 = mybir.dt.bfloat16


def build_kernel(nc: bass.Bass, Xap, DTap, Rap, OUTap, tc: tile.TileContext):
    from contextlib import ExitStack

    with ExitStack() as ctx:
        consts = ctx.enter_context(tc.tile_pool(name="consts", bufs=1))
        big = ctx.enter_context(tc.tile_pool(name="big", bufs=1))
        dsqp = ctx.enter_context(tc.tile_pool(name="dsqp", bufs=2))
        rbfp = ctx.enter_context(tc.tile_pool(name="rbfp", bufs=3))
        a3p = ctx.enter_context(tc.tile_pool(name="a3p", bufs=3))
        scrapp = ctx.enter_context(tc.tile_pool(name="scrapp", bufs=2))
        xnp = ctx.enter_context(tc.tile_pool(name="xnp", bufs=2))
        outp = ctx.enter_context(tc.tile_pool(name="outp", bufs=2))

        pa = ctx.enter_context(tc.tile_pool(name="pa", bufs=2, space="PSUM"))
        pecho = ctx.enter_context(tc.tile_pool(name="pecho", bufs=1, space="PSUM"))
        pss = ctx.enter_context(tc.tile_pool(name="pss", bufs=1, space="PSUM"))
        pt = ctx.enter_context(tc.tile_pool(name="pt", bufs=1, space="PSUM"))

        ident = consts.tile([P, P], F32)
        make_identity(nc, ident)
        ones = consts.tile([P, 1], F32)
        nc.vector.memset(ones, 1.0)

        # ---- bulk loads ------------------------------------------------
        # DT staged in f32 chunks, rounded on-chip to f32r (required for
        # f32r matmul operands), sumsq taken from the rounded values.
        DTbuf = big.tile([P, M], F32R)  # [d, m]
        Rbuf = big.tile([P, MT, d], F32)  # Rbuf[p, t, :] = r[t*128+p, :]
        Rr = Rap.rearrange("(p t) d -> p t d", t=MT)
        CH = M // 16  # 1024 cols / chunk (512 KB)
        CHT = MT // 16  # 8 m-tiles / chunk
        stagep = ctx.enter_context(tc.tile_pool(name="stagep", bufs=3))
        for c in range(16):
            stage = stagep.tile([P, CH], F32, tag="dstage")
            nc.sync.dma_start(out=stage, in_=DTap[:, ts(c, CH)])
            nc.vector.tensor_copy(DTbuf[:, ts(c, CH)], stage)
            nc.sync.dma_start(
                out=Rbuf[:, ts(c, CHT), :], in_=Rr[:, ts(c, CHT), :]
            )

        Xbuf = big.tile([P, NT, d], F32)
        Xr = Xap.rearrange("(i p) d -> p i d", p=P)
        nc.sync.dma_start(out=Xbuf[:, :, :], in_=Xr[:, :, :])

        # ---- X prep: normalize rows, transpose into XT [d, n_loc] f32 --
        ssx = consts.tile([P, NT], F32)
        sx = consts.tile([P, NT], F32)
        for i in range(NT):
            xscrap = scrapp.tile([P, d], BF16, tag="xscrap")
            nc.scalar.activation(
                out=xscrap,
                in_=Xbuf[:, i, :],
                func=mybir.ActivationFunctionType.Square,
                accum_out=ssx[:, i : i + 1],
            )
        nc.scalar.activation(out=sx, in_=ssx, func=mybir.ActivationFunctionType.Sqrt)
        nc.vector.reciprocal(out=sx, in_=sx)

        XT = consts.tile([P, N_LOC], F32R)  # [d, n], rounded for f32r matmul
        for i in range(NT):
            xn = xnp.tile([P, d], F32, tag="xn")
            nc.vector.tensor_scalar_mul(xn, Xbuf[:, i, :], sx[:, i : i + 1])
            ptile = pt.tile([P, P], F32, tag="pt")
            nc.tensor.transpose(ptile, xn, ident)
            nc.scalar.activation(
                out=XT[:, ts(i, P)],
                in_=ptile,
                func=mybir.ActivationFunctionType.Copy,
            )

        # ---- D row norms: ss_m = sum_d DT[:,m]^2 via Square + ones-matmul
        ss_ps = pss.tile([P, MT], F32)  # psum, col t = ss for m-tile t
        sd = consts.tile([P, MT], F32)
        for c in range(32):  # 512-col chunks
            dsq = dsqp.tile([P, 512], F32, tag="dsq")
            nc.scalar.activation(
                out=dsq,
                in_=DTbuf[:, ts(c, 512)],
                func=mybir.ActivationFunctionType.Square,
            )
            for k in range(4):
                t = 4 * c + k
                nc.tensor.matmul(
                    ss_ps[:, t : t + 1],
                    lhsT=dsq[:, ts(k, P)],
                    rhs=ones,
                    start=True,
                    stop=True,
                )
        for g in range(MT // GRP):
            nc.scalar.activation(
                out=sd[:, ts(g, GRP)],
                in_=ss_ps[:, ts(g, GRP)],
                func=mybir.ActivationFunctionType.Sqrt,
            )
            nc.vector.reciprocal(out=sd[:, ts(g, GRP)], in_=sd[:, ts(g, GRP)])

        # ---- main loop over m-tiles ------------------------------------
        echoT = pecho.tile([P, N_LOC], F32)  # [k, n] psum accumulator
        for t in range(MT):
            aT = pa.tile([P, N_LOC], F32, tag="aT")
            lhsT1 = DTbuf[:, ts(t, P)]
            for c in range(N_LOC // 512):
                nc.tensor.matmul(
                    aT[:, ts(c, 512)],
                    lhsT=lhsT1,
                    rhs=XT[:, ts(c, 512)],
                    start=True,
                    stop=True,
                )

            # fused cube with per-partition norm scale: a3 = (s_m * aT)^3
            a3 = a3p.tile([P, N_LOC], BF16, tag="a3")
            nc.vector._custom_dve(CUBE_OP, out=a3, in0=aT, s0=sd[:, t : t + 1])

            # r tile -> bf16
            rbf = rbfp.tile([P, P], BF16, tag="rbf")
            nc.scalar.activation(
                out=rbf, in_=Rbuf[:, t, :], func=mybir.ActivationFunctionType.Copy
            )

            # mm2: echoT[k, n] += r_tile.T @ a3
            for c in range(N_LOC // 512):
                nc.tensor.matmul(
                    echoT[:, ts(c, 512)],
                    lhsT=rbf,
                    rhs=a3[:, ts(c, 512)],
                    start=(t == 0),
                    stop=(t == MT - 1),
                )

        # ---- epilogue: transpose echoT -> OUT [n, k] --------------------
        echoS = consts.tile([P, N_LOC], F32)
        nc.scalar.activation(
            out=echoS, in_=echoT, func=mybir.ActivationFunctionType.Copy
        )
        for i in range(NT):
            ptile = pt.tile([P, P], F32, tag="pt")
            nc.tensor.transpose(ptile, echoS[:, ts(i, P)], ident)
            otile = outp.tile([P, P], F32, tag="otile")
            nc.vector.tensor_copy(otile, ptile)
            nc.sync.dma_start(out=OUTap[ts(i, P), :], in_=otile)


_COMPILED = None


def _get_compiled():
    global _COMPILED
    if _COMPILED is None:
        nc = bacc.Bacc(
            "TRN2",
            target_bir_lowering=False,
            debug=False,
            num_devices=1,
        )
        Xap = nc.dram_tensor("X", [N_LOC, d], F32, kind="ExternalInput").ap()
        DTap = nc.dram_tensor("DT", [d, M], F32, kind="ExternalInput").ap()
        Rap = nc.dram_tensor("RP", [M, d], F32, kind="ExternalInput").ap()
        OUTap = nc.dram_tensor("OUT", [N_LOC, d], F32, kind="ExternalOutput").ap()
        with tile.TileContext(nc) as tc:
            build_kernel(nc, Xap, DTap, Rap, OUTap, tc)
        nc.compile()
        _COMPILED = nc
    return _COMPILED


def kernel(X, D, r, _trace=False, _trace_kwargs=None):
    X = np.ascontiguousarray(np.asarray(X), dtype=np.float32)
    D = np.ascontiguousarray(np.asarray(D), dtype=np.float32)
    r = np.ascontiguousarray(np.asarray(r), dtype=np.float32)
    assert X.shape == (N, d) and D.shape == (M, d) and r.shape == (M, d)

    # host-side layout prep (no math): transpose D, tile-permute r
    DT = np.ascontiguousarray(D.T)  # [128, M]
    r_perm = np.ascontiguousarray(
        r.reshape(MT, P, d).transpose(1, 0, 2).reshape(M, d)
    )  # r_perm[p*128+t] = r[t*128+p]

    nc = _get_compiled()
    in_maps = [
        {
            "X": np.ascontiguousarray(X[c * N_LOC : (c + 1) * N_LOC]),
            "DT": DT,
            "RP": r_perm,
        }
        for c in range(NCORES)
    ]
    res = run_bass_kernel_spmd(
        nc,
        in_maps,
        core_ids=list(range(NCORES)),
        trace=_trace,
        **(_trace_kwargs or {}),
    )
    out = np.concatenate([res.results[c]["OUT"] for c in range(NCORES)], axis=0)
    if _trace:
        kernel._last_results = res
    return out
